# revision 1
# baseline (speedup 1.0000x reference)
"""Trainium2 Bass kernel for nn_LocalEnhancementModule (8-core SPMD, data-parallel over batch).

Per-sample computation (B=16, P=256 patches, D=4096, E=512):
    p      = patchify(x)                       [P, D]
    theta  = p @ theta_w + theta_b             [P, E]
    f      = p @ f_w + f_b                     [P, E]
    wgt    = softmax(theta @ f.T, axis=-1)     [P, P]
    g      = p @ g_w + g_b                     [P, D]
    out    = unpatchify(wgt[:,None,:] * g.reshape(P,C,P)) * scale + x

Sharding: 2 samples per core. Host pre-patchifies x and uploads a dense
fp16 pT (moving operand for theta/f, stationary for g), fp16 theta/f/g
weights, and fp32 p_nat for the residual. scale is folded into g_w on the
host. fp16 keeps ~tf32-class input precision (10-bit mantissa) at half the
HBM traffic of float32r, with fp32 PSUM accumulation throughout; softmax
runs in fp32.

Schedule: theta/f run k-outer into 8 PSUM banks (weights stream as full
[128,512] row-tiles); scores+softmax; then the g projection streams g_w
column slabs with 8 PSUM accumulators rotating over single-d rounds so two
rounds are always in flight. DMA issue is split across both HWDGE rings
(sync: pt16/theta_w/f_w/g_w; scalar: p_nat/out).
"""

import sys
import numpy as np

try:
    import concourse.bacc as bacc
except ImportError:  # pragma: no cover
    for _p in ("/opt/trn_rl_repo", "/root/.axon_site/_ro/trn_rl_repo"):
        if _p not in sys.path:
            sys.path.append(_p)
    import concourse.bacc as bacc
import concourse.mybir as mybir
import concourse.tile as tile
from concourse.bass_utils import run_bass_kernel_spmd

NCORES = 8
B, C, H, W = 16, 16, 256, 256
NPS, PH, PW = 16, 16, 16
P = NPS * NPS            # 256 patches
D = C * PH * PW          # 4096
E = 512
SPC = B // NCORES        # 2 samples per core
PP = SPC * P             # 512 patch rows per core
KT = D // 128            # 32 contraction tiles
ET = E // 128            # 4 embedding chunks
DCH = D // 512           # 8 column chunks for g
GRP = [(s, pc) for s in range(SPC) for pc in range(2)]

F32 = mybir.dt.float32
F16 = mybir.dt.float16

_built = {}
LAST_RESULTS = None  # stashed BassKernelResults for test harness introspection


def _build(with_tb, with_fb, with_gb):
    key = (with_tb, with_fb, with_gb)
    if key in _built:
        return _built[key]

    nc = bacc.Bacc("TRN2", num_devices=NCORES, debug=False)
    pt16_d = nc.dram_tensor("pt16", [D, PP], F16, kind="ExternalInput").ap()
    pnat_d = nc.dram_tensor("pnat", [PP, D], F32, kind="ExternalInput").ap()
    tw_d = nc.dram_tensor("tw", [D, E], F16, kind="ExternalInput").ap()
    fw_d = nc.dram_tensor("fw", [D, E], F16, kind="ExternalInput").ap()
    gw_d = nc.dram_tensor("gw", [D, D], F16, kind="ExternalInput").ap()
    tb_d = nc.dram_tensor("tb", [E, 1], F32, kind="ExternalInput").ap() if with_tb else None
    fb_d = nc.dram_tensor("fb", [E, 1], F32, kind="ExternalInput").ap() if with_fb else None
    gb_d = nc.dram_tensor("gb", [1, D], F32, kind="ExternalInput").ap() if with_gb else None
    out_d = nc.dram_tensor("out", [PP, D], F32, kind="ExternalOutput").ap()

    with tile.TileContext(nc) as tc:
        with tc.tile_pool(name="persist", bufs=1) as pp_, \
             tc.tile_pool(name="wstream", bufs=8) as wp, \
             tc.tile_pool(name="gstream", bufs=8) as gp, \
             tc.tile_pool(name="pnstream", bufs=4) as pnp, \
             tc.tile_pool(name="enh", bufs=6) as ep, \
             tc.tile_pool(name="sm", bufs=2) as smp:

            bias_sb = {0: [], 1: []}
            for wi, bd in ((0, tb_d), (1, fb_d)):
                if bd is None:
                    continue
                for e in range(ET):
                    bt = pp_.tile([128, 1], F32, name=f"bias_{wi}_{e}", tag=f"bias_{wi}_{e}")
                    nc.scalar.dma_start(out=bt[:, :], in_=bd[e * 128:(e + 1) * 128, :])
                    bias_sb[wi].append(bt)
            gb_sb = None
            if gb_d is not None:
                gb_sb = pp_.tile([1, D], F32, name="gb_sb", tag="gb_sb")
                nc.scalar.dma_start(out=gb_sb[:, :], in_=gb_d[:, :])

            # ---- theta / f projections, k-outer into 8 PSUM banks ----
            # projT[(w,e)] = [128(e), PP]  (thetaT / fT, fp16)
            pt16 = []
            with tc.tile_pool(name="psA", bufs=1, space="PSUM") as psA:
                ps_attn = {}
                for wi in (0, 1):
                    for e in range(ET):
                        ps_attn[(wi, e)] = psA.tile([128, PP], F32,
                                                    name=f"ps_attn_{wi}_{e}",
                                                    tag=f"attn_{wi}_{e}")
                for k in range(KT):
                    t16 = pp_.tile([128, PP], F16, name=f"pt16_{k}", tag=f"pt16_{k}")
                    nc.scalar.dma_start(out=t16[:, :], in_=pt16_d[k * 128:(k + 1) * 128, :])
                    pt16.append(t16)
                    for wi, wd in ((0, tw_d), (1, fw_d)):
                        wt = wp.tile([128, E], F16, name=f"wt_{wi}_{k}", tag="w")
                        nc.sync.dma_start(out=wt[:, :], in_=wd[k * 128:(k + 1) * 128, :])
                        for e in range(ET):
                            nc.tensor.matmul(ps_attn[(wi, e)][:, :],
                                             wt[:, e * 128:(e + 1) * 128],
                                             t16[:, :],
                                             start=(k == 0), stop=(k == KT - 1))
                proj_sb = {}
                for wi in (0, 1):
                    for e in range(ET):
                        sb = pp_.tile([128, PP], F16, name=f"proj_{wi}_{e}",
                                      tag=f"proj_{wi}_{e}")
                        if bias_sb[wi]:
                            nc.scalar.activation(sb[:, :], ps_attn[(wi, e)][:, :],
                                                 mybir.ActivationFunctionType.Identity,
                                                 bias=bias_sb[wi][e][:, :], scale=1.0)
                        elif e % 2 == 0:
                            nc.scalar.copy(sb[:, :], ps_attn[(wi, e)][:, :])
                        else:
                            nc.vector.tensor_copy(sb[:, :], ps_attn[(wi, e)][:, :])
                        proj_sb[(wi, e)] = sb

            # ---- scores + softmax per (sample, p-chunk) ----
            wgt = {}
            with tc.tile_pool(name="psB", bufs=1, space="PSUM") as psB:
                for (s, pc) in GRP:
                    sps = psB.tile([128, P], F32, name=f"ps_sc_{s}_{pc}", tag="sc", bufs=2)
                    col = s * P + pc * 128
                    for e in range(ET):
                        nc.tensor.matmul(sps[:, :],
                                         proj_sb[(0, e)][:, col:col + 128],
                                         proj_sb[(1, e)][:, s * P:(s + 1) * P],
                                         start=(e == 0), stop=(e == ET - 1))
                    mx = smp.tile([128, 1], F32, name=f"mx_{s}_{pc}", tag="mx")
                    nc.vector.tensor_reduce(out=mx[:, :], in_=sps[:, :],
                                            axis=mybir.AxisListType.X, op=mybir.AluOpType.max)
                    ngm = smp.tile([128, 1], F32, name=f"ngm_{s}_{pc}", tag="ngm")
                    nc.vector.tensor_scalar_mul(ngm[:, :], mx[:, :], -1.0)
                    ex = smp.tile([128, P], F32, name=f"ex_{s}_{pc}", tag="ex")
                    ssum = smp.tile([128, 1], F32, name=f"ssum_{s}_{pc}", tag="ssum")
                    nc.scalar.activation(ex[:, :], sps[:, :], mybir.ActivationFunctionType.Exp,
                                         bias=ngm[:, :], scale=1.0, accum_out=ssum[:, :])
                    rec = smp.tile([128, 1], F32, name=f"rec_{s}_{pc}", tag="rec")
                    nc.vector.reciprocal(rec[:, :], ssum[:, :])
                    wt_ = pp_.tile([128, P], F32, name=f"wgt_{s}_{pc}", tag=f"wgt_{s}_{pc}")
                    nc.vector.tensor_scalar_mul(wt_[:, :], ex[:, :], rec[:, :])
                    wgt[(s, pc)] = wt_

            # ---- g projection + gating + residual, single-d rounds, 2 in flight ----
            # Last round (d = DCH-1) uses gt tiles prefetched on the scalar ring
            # into a resident set during round DCH-3, and runs k-inner per group
            # so the final gating overlaps the remaining matmuls instead of
            # draining after the PE finishes.
            LAST = DCH - 1
            gs_last = []
            with tc.tile_pool(name="psC", bufs=1, space="PSUM") as psC:

                def gate_group(d, dcol, s, pc, g_ps):
                    row = s * P + pc * 128
                    if gb_sb is not None:
                        nc.vector.tensor_add(
                            g_ps[:, :], g_ps[:, :],
                            gb_sb[0:1, dcol:dcol + 512].partition_broadcast(128))
                    en = ep.tile([128, 512], F32, name=f"en_{d}_{s}_{pc}", tag="en")
                    nc.vector.tensor_mul(en[:, 0:256], g_ps[:, 0:256], wgt[(s, pc)][:, :])
                    nc.vector.tensor_mul(en[:, 256:512], g_ps[:, 256:512], wgt[(s, pc)][:, :])
                    pn = pnp.tile([128, 512], F32, name=f"pn_{d}_{s}_{pc}", tag="pn")
                    nc.scalar.dma_start(out=pn[:, :],
                                        in_=pnat_d[row:row + 128, dcol:dcol + 512])
                    nc.vector.tensor_add(en[:, :], en[:, :], pn[:, :])
                    nc.scalar.dma_start(out=out_d[row:row + 128, dcol:dcol + 512],
                                        in_=en[:, :])

                for d in range(LAST):
                    dcol = d * 512
                    gps = {}
                    for (s, pc) in GRP:
                        gps[(s, pc)] = psC.tile([128, 512], F32,
                                                name=f"ps_g_{d}_{s}_{pc}", tag="g", bufs=8)
                    for k in range(KT):
                        gt = gp.tile([128, 512], F16, name=f"gt_{d}_{k}", tag="gt")
                        nc.sync.dma_start(out=gt[:, :],
                                          in_=gw_d[k * 128:(k + 1) * 128, dcol:dcol + 512])
                        for (s, pc) in GRP:
                            col = s * P + pc * 128
                            nc.tensor.matmul(gps[(s, pc)][:, :],
                                             pt16[k][:, col:col + 128],
                                             gt[:, :],
                                             start=(k == 0), stop=(k == KT - 1))
                    for (s, pc) in GRP:
                        gate_group(d, dcol, s, pc, gps[(s, pc)])
                    if d == DCH - 3:
                        # prefetch the last round's g_w slab on the scalar ring
                        for k in range(KT):
                            gl = pp_.tile([128, 512], F16, name=f"gs_last_{k}",
                                          tag=f"gs_last_{k}")
                            nc.scalar.dma_start(
                                out=gl[:, :],
                                in_=gw_d[k * 128:(k + 1) * 128, LAST * 512:(LAST + 1) * 512])
                            gs_last.append(gl)

                dcol = LAST * 512
                for (s, pc) in GRP:
                    col = s * P + pc * 128
                    g_ps = psC.tile([128, 512], F32,
                                    name=f"ps_g_{LAST}_{s}_{pc}", tag="g", bufs=8)
                    for k in range(KT):
                        nc.tensor.matmul(g_ps[:, :], pt16[k][:, col:col + 128],
                                         gs_last[k][:, :],
                                         start=(k == 0), stop=(k == KT - 1))
                    gate_group(LAST, dcol, s, pc, g_ps)

    nc.compile()
    _built[key] = nc
    return nc


def kernel(**inputs):
    global LAST_RESULTS
    x = np.ascontiguousarray(inputs["x"], dtype=np.float32)
    tw = np.asarray(inputs["theta_w"], dtype=np.float32)
    fw = np.asarray(inputs["f_w"], dtype=np.float32)
    gw = np.asarray(inputs["g_w"], dtype=np.float32)
    tb = np.asarray(inputs["theta_b"], dtype=np.float32)
    fb = np.asarray(inputs["f_b"], dtype=np.float32)
    gb = np.asarray(inputs["g_b"], dtype=np.float32)
    scale = float(np.asarray(inputs["scale"], dtype=np.float32).reshape(-1)[0])

    with_tb = bool(np.any(tb))
    with_fb = bool(np.any(fb))
    with_gb = bool(np.any(gb))
    nc = _build(with_tb, with_fb, with_gb)

    # patchify: [B,C,H,W] -> [B,P,D] with D ordered (c, u, v)
    p = x.reshape(B, C, NPS, PH, NPS, PW).transpose(0, 2, 4, 1, 3, 5).reshape(B, P, D)
    tw16 = np.ascontiguousarray(tw).astype(np.float16)
    fw16 = np.ascontiguousarray(fw).astype(np.float16)
    gw16 = np.ascontiguousarray(scale * gw).astype(np.float16)
    in_maps = []
    for ci in range(NCORES):
        p2 = p[ci * SPC:(ci + 1) * SPC]                      # [SPC, P, D]
        pnat = np.ascontiguousarray(p2.reshape(PP, D), dtype=np.float32)
        pT16 = np.ascontiguousarray(p2.transpose(2, 0, 1).reshape(D, PP)).astype(np.float16)
        m = {"pt16": pT16, "pnat": pnat, "tw": tw16, "fw": fw16, "gw": gw16}
        if with_tb:
            m["tb"] = np.ascontiguousarray(tb.reshape(E, 1))
        if with_fb:
            m["fb"] = np.ascontiguousarray(fb.reshape(E, 1))
        if with_gb:
            m["gb"] = np.ascontiguousarray((scale * gb).reshape(1, D))
        in_maps.append(m)

    res = run_bass_kernel_spmd(nc, in_maps, core_ids=list(range(NCORES)))
    LAST_RESULTS = res
    o = np.concatenate([res.results[ci]["out"].reshape(SPC, P, D)
                        for ci in range(NCORES)], axis=0)     # [B, P, D]
    img = (o.reshape(B, NPS, NPS, C, PH, PW)
            .transpose(0, 3, 1, 4, 2, 5)
            .reshape(B, C, H, W))
    return np.ascontiguousarray(img, dtype=np.float32)



# revision 2
# speedup vs baseline: 1.3696x; 1.3696x over previous
"""Trainium2 Bass kernel for nn_LocalEnhancementModule (8-core SPMD, data-parallel over batch).

Per-sample computation (B=16, P=256 patches, D=4096, E=512):
    p      = patchify(x)                       [P, D]
    theta  = p @ theta_w + theta_b             [P, E]
    f      = p @ f_w + f_b                     [P, E]
    wgt    = softmax(theta @ f.T, axis=-1)     [P, P]
    g      = p @ g_w + g_b                     [P, D]
    out    = unpatchify(wgt[:,None,:] * g.reshape(P,C,P)) * scale + x

Sharding: 2 samples per core (PP=512 patch rows).

Precision/schedule: theta/f run in fp16 (the softmax scores need ~fp16
accuracy; measured fp8 scores push rel-err past tolerance). The dominant
g projection runs in fp8-e4m3 with MatmulPerfMode.DoubleRow (2 fp8
weights per PE cell, 256-deep contraction per matmul, ~2x throughput).
p is cast fp16->fp8 on-device (x16 scaling), g_w is quantized host-side
(x512 scaling, clipped to +-240); the 1/8192 unfold is folded into the
softmax reciprocal. Residual and output ride fp16 DMA (x dominates the
output norm; fp16 rounding is ~2e-4 rel). PSUM accumulates fp32
throughout.

DMA: k-quad interleaved layouts give 2-4KB contiguous rows per transfer.
Traffic is split over three rings: sync (tf even-k, gw8 even rounds),
gpsimd (tf odd-k, gw8 odd rounds), scalar (p tiles, residual, output).
"""

import sys
import numpy as np

try:
    import concourse.bacc as bacc
except ImportError:  # pragma: no cover
    for _p in ("/opt/trn_rl_repo", "/root/.axon_site/_ro/trn_rl_repo"):
        if _p not in sys.path:
            sys.path.append(_p)
    import concourse.bacc as bacc
import concourse.mybir as mybir
import concourse.tile as tile
from concourse.bass_utils import run_bass_kernel_spmd

NCORES = 8
B, C, H, W = 16, 16, 256, 256
NPS, PH, PW = 16, 16, 16
P = NPS * NPS            # 256 patches
D = C * PH * PW          # 4096
E = 512
SPC = B // NCORES        # 2 samples per core
PP = SPC * P             # 512 patch rows per core
KT = D // 128            # 32 contraction tiles of 128
KT4 = D // 512           # 8 k-quad tiles (4 x 128)
ET = E // 128            # 4 embedding chunks
DCH = D // 512           # 8 column rounds for g
DP = DCH // 2            # 4 column-pair rounds (1024-wide output writes)
GRP = [(s, pc) for s in range(SPC) for pc in range(2)]

SF_P = 16.0              # fp8 scale for p
SF_G = 512.0             # fp8 scale for g_w
UNSCALE = 1.0 / (SF_P * SF_G)

F32 = mybir.dt.float32
F16 = mybir.dt.float16
F8 = mybir.dt.float8e4
DR = mybir.MatmulPerfMode.DoubleRow

_built = {}
LAST_RESULTS = None  # stashed BassKernelResults for test harness introspection


def _build(with_tb, with_fb, with_gb):
    key = (with_tb, with_fb, with_gb)
    if key in _built:
        return _built[key]

    nc = bacc.Bacc("TRN2", num_devices=NCORES, debug=False)
    # ptq: pT fp16, k-quad interleaved: row kq*128+part, col ks*PP+pp,
    #      element = pT[k=kq*512+ks*128+part, pp]
    ptq_d = nc.dram_tensor("ptq", [KT4 * 128, 4 * PP], F16, kind="ExternalInput").ap()
    pnat_d = nc.dram_tensor("pnat", [PP, D], F16, kind="ExternalInput").ap()
    # tf: concat(theta_w | f_w) columns, [D, 2E] fp16
    tf_d = nc.dram_tensor("tf", [D, 2 * E], F16, kind="ExternalInput").ap()
    # gw8: fp8 g_w * scale * SF_G; row kq*128+part, col dch*2048+ks*512+n,
    #      element = gw[k=kq*512+ks*128+part, d=dch*512+n]
    gw8_d = nc.dram_tensor("gw8", [KT4 * 128, DCH * 2048], F8, kind="ExternalInput").ap()
    tb_d = nc.dram_tensor("tb", [E, 1], F32, kind="ExternalInput").ap() if with_tb else None
    fb_d = nc.dram_tensor("fb", [E, 1], F32, kind="ExternalInput").ap() if with_fb else None
    gb_d = nc.dram_tensor("gb", [1, D], F32, kind="ExternalInput").ap() if with_gb else None
    out_d = nc.dram_tensor("out", [PP, D], F16, kind="ExternalOutput").ap()

    with tile.TileContext(nc) as tc:
        with tc.tile_pool(name="persist", bufs=1) as pp_, \
             tc.tile_pool(name="ptstream", bufs=3) as pqp, \
             tc.tile_pool(name="wstream", bufs=6) as wp, \
             tc.tile_pool(name="gstream", bufs=16) as gp, \
             tc.tile_pool(name="pnstream", bufs=8) as pnp, \
             tc.tile_pool(name="enh", bufs=8) as ep, \
             tc.tile_pool(name="sm", bufs=2) as smp:

            bias_sb = {0: [], 1: []}
            for wi, bd in ((0, tb_d), (1, fb_d)):
                if bd is None:
                    continue
                for e in range(ET):
                    bt = pp_.tile([128, 1], F32, name=f"bias_{wi}_{e}", tag=f"bias_{wi}_{e}")
                    nc.scalar.dma_start(out=bt[:, :], in_=bd[e * 128:(e + 1) * 128, :])
                    bias_sb[wi].append(bt)
            gb_sb = None
            if gb_d is not None:
                gb_sb = pp_.tile([1, D], F32, name="gb_sb", tag="gb_sb")
                nc.scalar.dma_start(out=gb_sb[:, :], in_=gb_d[:, :])

            # ---- theta / f projections (fp16), k-outer into 8 PSUM banks ----
            pt8_sb = []
            with tc.tile_pool(name="psA", bufs=1, space="PSUM") as psA:
                ps_attn = {}
                for wi in (0, 1):
                    for e in range(ET):
                        ps_attn[(wi, e)] = psA.tile([128, PP], F32,
                                                    name=f"ps_attn_{wi}_{e}",
                                                    tag=f"attn_{wi}_{e}")
                for kq in range(KT4):
                    ptq_t = pqp.tile([128, 4, PP], F16, name=f"ptq_{kq}", tag="ptq")
                    nc.scalar.dma_start(out=ptq_t[:, :, :],
                                        in_=ptq_d[kq * 128:(kq + 1) * 128, :])
                    p8 = pp_.tile([128, 4, PP], F8, name=f"pt8_{kq}", tag=f"pt8_{kq}")
                    nc.vector.tensor_scalar_mul(p8[:, :, :], ptq_t[:, :, :], SF_P)
                    pt8_sb.append(p8)
                    for ks in range(4):
                        k = kq * 4 + ks
                        tf_t = wp.tile([128, 2 * E], F16, name=f"tf_{k}", tag="w")
                        eng = nc.sync if ks % 2 == 0 else nc.gpsimd
                        eng.dma_start(out=tf_t[:, :], in_=tf_d[k * 128:(k + 1) * 128, :])
                        for wi in (0, 1):
                            for e in range(ET):
                                nc.tensor.matmul(
                                    ps_attn[(wi, e)][:, :],
                                    tf_t[:, wi * E + e * 128: wi * E + (e + 1) * 128],
                                    ptq_t[:, ks, :],
                                    start=(k == 0), stop=(k == KT - 1))
                proj_sb = {}
                for wi in (0, 1):
                    for e in range(ET):
                        sb = pp_.tile([128, PP], F16, name=f"proj_{wi}_{e}",
                                      tag=f"proj_{wi}_{e}")
                        if bias_sb[wi]:
                            nc.scalar.activation(sb[:, :], ps_attn[(wi, e)][:, :],
                                                 mybir.ActivationFunctionType.Identity,
                                                 bias=bias_sb[wi][e][:, :], scale=1.0)
                        elif e % 2 == 0:
                            nc.scalar.copy(sb[:, :], ps_attn[(wi, e)][:, :])
                        else:
                            nc.vector.tensor_copy(sb[:, :], ps_attn[(wi, e)][:, :])
                        proj_sb[(wi, e)] = sb

            # ---- scores + softmax per (sample, p-chunk); fold fp8 unscale ----
            wgt = {}
            with tc.tile_pool(name="psB", bufs=1, space="PSUM") as psB:
                for (s, pc) in GRP:
                    sps = psB.tile([128, P], F32, name=f"ps_sc_{s}_{pc}", tag="sc", bufs=2)
                    col = s * P + pc * 128
                    for e in range(ET):
                        nc.tensor.matmul(sps[:, :],
                                         proj_sb[(0, e)][:, col:col + 128],
                                         proj_sb[(1, e)][:, s * P:(s + 1) * P],
                                         start=(e == 0), stop=(e == ET - 1))
                    mx = smp.tile([128, 1], F32, name=f"mx_{s}_{pc}", tag="mx")
                    nc.vector.tensor_reduce(out=mx[:, :], in_=sps[:, :],
                                            axis=mybir.AxisListType.X, op=mybir.AluOpType.max)
                    ngm = smp.tile([128, 1], F32, name=f"ngm_{s}_{pc}", tag="ngm")
                    nc.vector.tensor_scalar_mul(ngm[:, :], mx[:, :], -1.0)
                    ex = smp.tile([128, P], F32, name=f"ex_{s}_{pc}", tag="ex")
                    ssum = smp.tile([128, 1], F32, name=f"ssum_{s}_{pc}", tag="ssum")
                    nc.scalar.activation(ex[:, :], sps[:, :], mybir.ActivationFunctionType.Exp,
                                         bias=ngm[:, :], scale=1.0, accum_out=ssum[:, :])
                    ssc = smp.tile([128, 1], F32, name=f"ssc_{s}_{pc}", tag="ssc")
                    nc.vector.tensor_scalar_mul(ssc[:, :], ssum[:, :], float(SF_P * SF_G))
                    rec = smp.tile([128, 1], F32, name=f"rec_{s}_{pc}", tag="rec")
                    nc.vector.reciprocal(rec[:, :], ssc[:, :])
                    wt_ = pp_.tile([128, P], F32, name=f"wgt_{s}_{pc}", tag=f"wgt_{s}_{pc}")
                    nc.vector.tensor_scalar_mul(wt_[:, :], ex[:, :], rec[:, :])
                    wgt[(s, pc)] = wt_

            # ---- g projection (fp8 DoubleRow) + gating + residual ----
            with tc.tile_pool(name="psC", bufs=1, space="PSUM") as psC:
                for dp in range(DP):
                    en16 = {}
                    pn16 = {}
                    for gi, (s, pc) in enumerate(GRP):
                        en16[gi] = ep.tile([128, 1024], F16, name=f"en_{dp}_{gi}", tag="en")
                        pn16[gi] = pnp.tile([128, 1024], F16, name=f"pn_{dp}_{gi}", tag="pn")
                        row = s * P + pc * 128
                        nc.scalar.dma_start(
                            out=pn16[gi][:, :],
                            in_=pnat_d[row:row + 128, dp * 1024:(dp + 1) * 1024])
                    for sub in range(2):
                        dch = dp * 2 + sub
                        gps = {}
                        for gi, (s, pc) in enumerate(GRP):
                            gps[gi] = psC.tile([128, 512], F32,
                                               name=f"ps_g_{dch}_{gi}", tag="g", bufs=8)
                        for kq in range(KT4):
                            gt = gp.tile([128, 4, 512], F8, name=f"gt_{dch}_{kq}", tag="gt")
                            eng = nc.sync if dch % 2 == 0 else nc.gpsimd
                            eng.dma_start(
                                out=gt[:, :, :],
                                in_=gw8_d[kq * 128:(kq + 1) * 128,
                                          dch * 2048:(dch + 1) * 2048])
                            for gi, (s, pc) in enumerate(GRP):
                                col = s * P + pc * 128
                                for pr in range(2):
                                    nc.tensor.matmul(
                                        gps[gi][:, :],
                                        pt8_sb[kq][:, 2 * pr:2 * pr + 2, col:col + 128],
                                        gt[:, 2 * pr:2 * pr + 2, :],
                                        start=(kq == 0 and pr == 0),
                                        stop=(kq == KT4 - 1 and pr == 1),
                                        perf_mode=DR)
                        for gi, (s, pc) in enumerate(GRP):
                            if gb_sb is not None:
                                nc.vector.tensor_add(
                                    gps[gi][:, :], gps[gi][:, :],
                                    gb_sb[0:1, dch * 512:(dch + 1) * 512]
                                    .partition_broadcast(128))
                            base = sub * 512
                            nc.vector.tensor_mul(en16[gi][:, base:base + 256],
                                                 gps[gi][:, 0:256], wgt[(s, pc)][:, :])
                            nc.vector.tensor_mul(en16[gi][:, base + 256:base + 512],
                                                 gps[gi][:, 256:512], wgt[(s, pc)][:, :])
                            nc.vector.tensor_add(en16[gi][:, base:base + 512],
                                                 en16[gi][:, base:base + 512],
                                                 pn16[gi][:, base:base + 512])
                            if sub == 1:
                                row = s * P + pc * 128
                                nc.scalar.dma_start(
                                    out=out_d[row:row + 128, dp * 1024:(dp + 1) * 1024],
                                    in_=en16[gi][:, :])

    nc.compile()
    _built[key] = nc
    return nc


def kernel(**inputs):
    global LAST_RESULTS
    x = np.ascontiguousarray(inputs["x"], dtype=np.float32)
    tw = np.asarray(inputs["theta_w"], dtype=np.float32)
    fw = np.asarray(inputs["f_w"], dtype=np.float32)
    gw = np.asarray(inputs["g_w"], dtype=np.float32)
    tb = np.asarray(inputs["theta_b"], dtype=np.float32)
    fb = np.asarray(inputs["f_b"], dtype=np.float32)
    gb = np.asarray(inputs["g_b"], dtype=np.float32)
    scale = float(np.asarray(inputs["scale"], dtype=np.float32).reshape(-1)[0])

    with_tb = bool(np.any(tb))
    with_fb = bool(np.any(fb))
    with_gb = bool(np.any(gb))
    nc = _build(with_tb, with_fb, with_gb)

    F8NP = mybir.dt.np(F8)
    # patchify: [B,C,H,W] -> [B,P,D] with D ordered (c, u, v)
    p = x.reshape(B, C, NPS, PH, NPS, PW).transpose(0, 2, 4, 1, 3, 5).reshape(B, P, D)
    tf16 = np.ascontiguousarray(np.concatenate([tw, fw], axis=1)).astype(np.float16)
    # gw8: [kq, ks, part, dch, n] -> [kq, part, dch, ks, n]
    gq = np.clip(gw * (scale * SF_G), -240.0, 240.0).astype(F8NP)
    gw8 = np.ascontiguousarray(
        gq.reshape(KT4, 4, 128, DCH, 512).transpose(0, 2, 3, 1, 4)
          .reshape(KT4 * 128, DCH * 2048))
    in_maps = []
    for ci in range(NCORES):
        p2 = p[ci * SPC:(ci + 1) * SPC]                      # [SPC, P, D]
        pnat = p2.reshape(PP, D).astype(np.float16)
        pT = p2.transpose(2, 0, 1).reshape(D, PP)
        ptq = np.ascontiguousarray(
            pT.reshape(KT4, 4, 128, PP).transpose(0, 2, 1, 3)
              .reshape(KT4 * 128, 4 * PP)).astype(np.float16)
        m = {"ptq": ptq, "pnat": pnat, "tf": tf16, "gw8": gw8}
        if with_tb:
            m["tb"] = np.ascontiguousarray(tb.reshape(E, 1))
        if with_fb:
            m["fb"] = np.ascontiguousarray(fb.reshape(E, 1))
        if with_gb:
            m["gb"] = np.ascontiguousarray((gb * (scale * SF_P * SF_G)).reshape(1, D))
        in_maps.append(m)

    res = run_bass_kernel_spmd(nc, in_maps, core_ids=list(range(NCORES)))
    LAST_RESULTS = res
    o = np.concatenate([np.asarray(res.results[ci]["out"], dtype=np.float32)
                        .reshape(SPC, P, D)
                        for ci in range(NCORES)], axis=0)     # [B, P, D]
    img = (o.reshape(B, NPS, NPS, C, PH, PW)
            .transpose(0, 3, 1, 4, 2, 5)
            .reshape(B, C, H, W))
    return np.ascontiguousarray(img, dtype=np.float32)


# revision 5
# speedup vs baseline: 1.5958x; 1.1652x over previous
"""Trainium2 Bass kernel for nn_LocalEnhancementModule (8-core SPMD, data-parallel over batch).

Per-sample computation (B=16, P=256 patches, D=4096, E=512):
    p      = patchify(x)                       [P, D]
    theta  = p @ theta_w + theta_b             [P, E]
    f      = p @ f_w + f_b                     [P, E]
    wgt    = softmax(theta @ f.T, axis=-1)     [P, P]
    g      = p @ g_w + g_b                     [P, D]
    out    = unpatchify(wgt[:,None,:] * g.reshape(P,C,P)) * scale + x

Sharding: 2 samples per core (PP=512 patch rows).

Precision/schedule: theta/f run in fp16 (the softmax scores need ~fp16
accuracy; measured fp8 scores push rel-err past tolerance). The dominant
g projection runs in fp8-e4m3 with MatmulPerfMode.DoubleRow (2 fp8
weights per PE cell, 256-deep contraction per matmul, ~2x throughput).
p is cast fp16->fp8 on-device (x16 scaling), g_w is quantized host-side
(x512 scaling, clipped to +-240); the 1/8192 unfold is folded into the
softmax reciprocal. Residual and output ride fp16 DMA (x dominates the
output norm; fp16 rounding is ~2e-4 rel). PSUM accumulates fp32
throughout.

DMA: k-quad interleaved layouts give 2-4KB contiguous rows per transfer.
Traffic is split over three rings: sync (tf even-k, gw8 even rounds),
gpsimd (tf odd-k, gw8 odd rounds), scalar (p tiles, residual, output).
"""

import sys
import numpy as np

try:
    import concourse.bacc as bacc
except ImportError:  # pragma: no cover
    for _p in ("/opt/trn_rl_repo", "/root/.axon_site/_ro/trn_rl_repo"):
        if _p not in sys.path:
            sys.path.append(_p)
    import concourse.bacc as bacc
import concourse.mybir as mybir
import concourse.tile as tile
from concourse.bass_utils import run_bass_kernel_spmd

NCORES = 8
B, C, H, W = 16, 16, 256, 256
NPS, PH, PW = 16, 16, 16
P = NPS * NPS            # 256 patches
D = C * PH * PW          # 4096
E = 512
SPC = B // NCORES        # 2 samples per core
PP = SPC * P             # 512 patch rows per core
KT = D // 128            # 32 contraction tiles of 128
KT4 = D // 512           # 8 k-quad tiles (4 x 128)
ET = E // 128            # 4 embedding chunks
DCH = D // 512           # 8 column rounds for g
DP = DCH // 2            # 4 column-pair rounds (1024-wide output writes)
GRP = [(s, pc) for s in range(SPC) for pc in range(2)]

SF_P = 16.0              # fp8 scale for p
SF_G = 512.0             # fp8 scale for g_w
UNSCALE = 1.0 / (SF_P * SF_G)

F32 = mybir.dt.float32
F16 = mybir.dt.float16
F8 = mybir.dt.float8e4
DR = mybir.MatmulPerfMode.DoubleRow

_built = {}
LAST_RESULTS = None  # stashed BassKernelResults for test harness introspection


def _build(with_tb, with_fb, with_gb):
    key = (with_tb, with_fb, with_gb)
    if key in _built:
        return _built[key]

    nc = bacc.Bacc("TRN2", num_devices=NCORES, debug=False)
    # ptq: pT fp16, k-quad interleaved: row kq*128+part, col ks*PP+pp,
    #      element = pT[k=kq*512+ks*128+part, pp]
    ptq_d = nc.dram_tensor("ptq", [KT4 * 128, 4 * PP], F16, kind="ExternalInput").ap()
    pnat_d = nc.dram_tensor("pnat", [PP, D], F16, kind="ExternalInput").ap()
    # tf: concat(theta_w | f_w) columns, [D, 2E] fp16
    tf_d = nc.dram_tensor("tf", [D, 2 * E], F16, kind="ExternalInput").ap()
    # gw8: fp8 g_w * scale * SF_G; row kq*128+part, col dch*2048+ks*512+n,
    #      element = gw[k=kq*512+ks*128+part, d=dch*512+n]
    gw8_d = nc.dram_tensor("gw8", [KT4 * 128, DCH * 2048], F8, kind="ExternalInput").ap()
    tb_d = nc.dram_tensor("tb", [E, 1], F32, kind="ExternalInput").ap() if with_tb else None
    fb_d = nc.dram_tensor("fb", [E, 1], F32, kind="ExternalInput").ap() if with_fb else None
    gb_d = nc.dram_tensor("gb", [1, D], F32, kind="ExternalInput").ap() if with_gb else None
    out_d = nc.dram_tensor("out", [PP, D], F16, kind="ExternalOutput").ap()

    with tile.TileContext(nc) as tc:
        with tc.tile_pool(name="persist", bufs=1) as pp_, \
             tc.tile_pool(name="ptstream", bufs=4) as pqp, \
             tc.tile_pool(name="wstream", bufs=12) as wp, \
             tc.tile_pool(name="gstream", bufs=16) as gp, \
             tc.tile_pool(name="pnstream", bufs=8) as pnp, \
             tc.tile_pool(name="enh", bufs=8) as ep, \
             tc.tile_pool(name="sm", bufs=2) as smp:

            bias_sb = {0: [], 1: []}
            for wi, bd in ((0, tb_d), (1, fb_d)):
                if bd is None:
                    continue
                for e in range(ET):
                    bt = pp_.tile([128, 1], F32, name=f"bias_{wi}_{e}", tag=f"bias_{wi}_{e}")
                    nc.scalar.dma_start(out=bt[:, :], in_=bd[e * 128:(e + 1) * 128, :])
                    bias_sb[wi].append(bt)
            gb_sb = None
            if gb_d is not None:
                gb_sb = pp_.tile([1, D], F32, name="gb_sb", tag="gb_sb")
                nc.scalar.dma_start(out=gb_sb[:, :], in_=gb_d[:, :])

            # ---- theta / f projections (fp16), k-outer into 8 PSUM banks ----
            pt8_sb = []
            with tc.tile_pool(name="psA", bufs=1, space="PSUM") as psA:
                ps_attn = {}
                for wi in (0, 1):
                    for e in range(ET):
                        ps_attn[(wi, e)] = psA.tile([128, PP], F32,
                                                    name=f"ps_attn_{wi}_{e}",
                                                    tag=f"attn_{wi}_{e}")
                for kq in range(KT4):
                    ptq_t = pqp.tile([128, 4, PP], F16, name=f"ptq_{kq}", tag="ptq")
                    p8 = pp_.tile([128, 4, PP], F8, name=f"pt8_{kq}", tag=f"pt8_{kq}")
                    for ks in range(4):
                        # per-ks DMA + cast so the first matmul starts after
                        # 128KB rather than a full 512KB tile
                        nc.scalar.dma_start(
                            out=ptq_t[:, ks, :],
                            in_=ptq_d[kq * 128:(kq + 1) * 128,
                                      ks * PP:(ks + 1) * PP])
                        nc.vector.tensor_scalar_mul(p8[:, ks, :], ptq_t[:, ks, :], SF_P)
                    pt8_sb.append(p8)
                    for ks in range(4):
                        k = kq * 4 + ks
                        tf_t = wp.tile([128, 2 * E], F16, name=f"tf_{k}", tag="w")
                        eng = nc.sync if ks % 2 == 0 else nc.gpsimd
                        eng.dma_start(out=tf_t[:, :], in_=tf_d[k * 128:(k + 1) * 128, :])
                        for wi in (0, 1):
                            for e in range(ET):
                                nc.tensor.matmul(
                                    ps_attn[(wi, e)][:, :],
                                    tf_t[:, wi * E + e * 128: wi * E + (e + 1) * 128],
                                    ptq_t[:, ks, :],
                                    start=(k == 0), stop=(k == KT - 1))
                proj_sb = {}
                for wi in (0, 1):
                    for e in range(ET):
                        sb = pp_.tile([128, PP], F16, name=f"proj_{wi}_{e}",
                                      tag=f"proj_{wi}_{e}")
                        if bias_sb[wi]:
                            nc.scalar.activation(sb[:, :], ps_attn[(wi, e)][:, :],
                                                 mybir.ActivationFunctionType.Identity,
                                                 bias=bias_sb[wi][e][:, :], scale=1.0)
                        elif e % 2 == 0:
                            nc.scalar.copy(sb[:, :], ps_attn[(wi, e)][:, :])
                        else:
                            nc.vector.tensor_copy(sb[:, :], ps_attn[(wi, e)][:, :])
                        proj_sb[(wi, e)] = sb

            # ---- scores + softmax per (sample, p-chunk); fold fp8 unscale ----
            wgt = {}
            with tc.tile_pool(name="psB", bufs=1, space="PSUM") as psB:
                for (s, pc) in GRP:
                    sps = psB.tile([128, P], F32, name=f"ps_sc_{s}_{pc}", tag="sc", bufs=2)
                    col = s * P + pc * 128
                    for e in range(ET):
                        nc.tensor.matmul(sps[:, :],
                                         proj_sb[(0, e)][:, col:col + 128],
                                         proj_sb[(1, e)][:, s * P:(s + 1) * P],
                                         start=(e == 0), stop=(e == ET - 1))
                    mx = smp.tile([128, 1], F32, name=f"mx_{s}_{pc}", tag="mx")
                    nc.vector.tensor_reduce(out=mx[:, :], in_=sps[:, :],
                                            axis=mybir.AxisListType.X, op=mybir.AluOpType.max)
                    ngm = smp.tile([128, 1], F32, name=f"ngm_{s}_{pc}", tag="ngm")
                    nc.vector.tensor_scalar_mul(ngm[:, :], mx[:, :], -1.0)
                    ex = smp.tile([128, P], F32, name=f"ex_{s}_{pc}", tag="ex")
                    ssum = smp.tile([128, 1], F32, name=f"ssum_{s}_{pc}", tag="ssum")
                    nc.scalar.activation(ex[:, :], sps[:, :], mybir.ActivationFunctionType.Exp,
                                         bias=ngm[:, :], scale=1.0, accum_out=ssum[:, :])
                    ssc = smp.tile([128, 1], F32, name=f"ssc_{s}_{pc}", tag="ssc")
                    nc.vector.tensor_scalar_mul(ssc[:, :], ssum[:, :], float(SF_P * SF_G))
                    rec = smp.tile([128, 1], F32, name=f"rec_{s}_{pc}", tag="rec")
                    nc.vector.reciprocal(rec[:, :], ssc[:, :])
                    wt_ = pp_.tile([128, P], F32, name=f"wgt_{s}_{pc}", tag=f"wgt_{s}_{pc}")
                    nc.vector.tensor_scalar_mul(wt_[:, :], ex[:, :], rec[:, :])
                    wgt[(s, pc)] = wt_

            # ---- g projection (fp8 DoubleRow) + gating + residual ----
            OUT_ENG = [nc.scalar, nc.sync, nc.gpsimd, nc.scalar]
            with tc.tile_pool(name="psC", bufs=1, space="PSUM") as psC:
                for dp in range(DP):
                    en16 = {}
                    pn16 = {}
                    for gi, (s, pc) in enumerate(GRP):
                        en16[gi] = ep.tile([128, 1024], F16, name=f"en_{dp}_{gi}", tag="en")
                        pn16[gi] = pnp.tile([128, 1024], F16, name=f"pn_{dp}_{gi}", tag="pn")
                        row = s * P + pc * 128
                        nc.scalar.dma_start(
                            out=pn16[gi][:, :],
                            in_=pnat_d[row:row + 128, dp * 1024:(dp + 1) * 1024])

                    def gate(dch, gi, s, pc, g_ps, dp=dp, en16=en16, pn16=pn16):
                        if gb_sb is not None:
                            nc.vector.tensor_add(
                                g_ps[:, :], g_ps[:, :],
                                gb_sb[0:1, dch * 512:(dch + 1) * 512]
                                .partition_broadcast(128))
                        base = (dch % 2) * 512
                        nc.vector.tensor_mul(en16[gi][:, base:base + 256],
                                             g_ps[:, 0:256], wgt[(s, pc)][:, :])
                        nc.vector.tensor_mul(en16[gi][:, base + 256:base + 512],
                                             g_ps[:, 256:512], wgt[(s, pc)][:, :])
                        nc.vector.tensor_add(en16[gi][:, base:base + 512],
                                             en16[gi][:, base:base + 512],
                                             pn16[gi][:, base:base + 512])
                        if dch % 2 == 1:
                            row = s * P + pc * 128
                            eng = OUT_ENG[gi] if dp == DP - 1 else nc.scalar
                            eng.dma_start(
                                out=out_d[row:row + 128, dp * 1024:(dp + 1) * 1024],
                                in_=en16[gi][:, :])

                    for sub in range(2):
                        dch = dp * 2 + sub
                        last = (dp == DP - 1 and sub == 1)
                        gts = []
                        for kq in range(KT4):
                            gt = gp.tile([128, 4, 512], F8, name=f"gt_{dch}_{kq}", tag="gt")
                            eng = nc.sync if dch % 2 == 0 else nc.gpsimd
                            eng.dma_start(
                                out=gt[:, :, :],
                                in_=gw8_d[kq * 128:(kq + 1) * 128,
                                          dch * 2048:(dch + 1) * 2048])
                            gts.append(gt)
                        if not last:
                            gps = {}
                            for gi, (s, pc) in enumerate(GRP):
                                gps[gi] = psC.tile([128, 512], F32,
                                                   name=f"ps_g_{dch}_{gi}", tag="g", bufs=8)
                            for kq in range(KT4):
                                for gi, (s, pc) in enumerate(GRP):
                                    col = s * P + pc * 128
                                    for pr in range(2):
                                        nc.tensor.matmul(
                                            gps[gi][:, :],
                                            pt8_sb[kq][:, 2 * pr:2 * pr + 2, col:col + 128],
                                            gts[kq][:, 2 * pr:2 * pr + 2, :],
                                            start=(kq == 0 and pr == 0),
                                            stop=(kq == KT4 - 1 and pr == 1),
                                            perf_mode=DR)
                            for gi, (s, pc) in enumerate(GRP):
                                gate(dch, gi, s, pc, gps[gi])
                        else:
                            # final round k-inner per group: gating/writes of
                            # earlier groups overlap the remaining matmuls
                            for gi, (s, pc) in enumerate(GRP):
                                col = s * P + pc * 128
                                g_ps = psC.tile([128, 512], F32,
                                                name=f"ps_g_{dch}_{gi}", tag="g", bufs=8)
                                for kq in range(KT4):
                                    for pr in range(2):
                                        nc.tensor.matmul(
                                            g_ps[:, :],
                                            pt8_sb[kq][:, 2 * pr:2 * pr + 2, col:col + 128],
                                            gts[kq][:, 2 * pr:2 * pr + 2, :],
                                            start=(kq == 0 and pr == 0),
                                            stop=(kq == KT4 - 1 and pr == 1),
                                            perf_mode=DR)
                                gate(dch, gi, s, pc, g_ps)

    nc.compile()
    _built[key] = nc
    return nc


def kernel(**inputs):
    global LAST_RESULTS
    x = np.ascontiguousarray(inputs["x"], dtype=np.float32)
    tw = np.asarray(inputs["theta_w"], dtype=np.float32)
    fw = np.asarray(inputs["f_w"], dtype=np.float32)
    gw = np.asarray(inputs["g_w"], dtype=np.float32)
    tb = np.asarray(inputs["theta_b"], dtype=np.float32)
    fb = np.asarray(inputs["f_b"], dtype=np.float32)
    gb = np.asarray(inputs["g_b"], dtype=np.float32)
    scale = float(np.asarray(inputs["scale"], dtype=np.float32).reshape(-1)[0])

    with_tb = bool(np.any(tb))
    with_fb = bool(np.any(fb))
    with_gb = bool(np.any(gb))
    nc = _build(with_tb, with_fb, with_gb)

    F8NP = mybir.dt.np(F8)
    # patchify: [B,C,H,W] -> [B,P,D] with D ordered (c, u, v)
    p = x.reshape(B, C, NPS, PH, NPS, PW).transpose(0, 2, 4, 1, 3, 5).reshape(B, P, D)
    tf16 = np.ascontiguousarray(np.concatenate([tw, fw], axis=1)).astype(np.float16)
    # gw8: [kq, ks, part, dch, n] -> [kq, part, dch, ks, n]
    gq = np.clip(gw * (scale * SF_G), -240.0, 240.0).astype(F8NP)
    gw8 = np.ascontiguousarray(
        gq.reshape(KT4, 4, 128, DCH, 512).transpose(0, 2, 3, 1, 4)
          .reshape(KT4 * 128, DCH * 2048))
    in_maps = []
    for ci in range(NCORES):
        p2 = p[ci * SPC:(ci + 1) * SPC]                      # [SPC, P, D]
        pnat = p2.reshape(PP, D).astype(np.float16)
        pT = p2.transpose(2, 0, 1).reshape(D, PP)
        ptq = np.ascontiguousarray(
            pT.reshape(KT4, 4, 128, PP).transpose(0, 2, 1, 3)
              .reshape(KT4 * 128, 4 * PP)).astype(np.float16)
        m = {"ptq": ptq, "pnat": pnat, "tf": tf16, "gw8": gw8}
        if with_tb:
            m["tb"] = np.ascontiguousarray(tb.reshape(E, 1))
        if with_fb:
            m["fb"] = np.ascontiguousarray(fb.reshape(E, 1))
        if with_gb:
            m["gb"] = np.ascontiguousarray((gb * (scale * SF_P * SF_G)).reshape(1, D))
        in_maps.append(m)

    res = run_bass_kernel_spmd(nc, in_maps, core_ids=list(range(NCORES)))
    LAST_RESULTS = res
    o = np.concatenate([np.asarray(res.results[ci]["out"], dtype=np.float32)
                        .reshape(SPC, P, D)
                        for ci in range(NCORES)], axis=0)     # [B, P, D]
    img = (o.reshape(B, NPS, NPS, C, PH, PW)
            .transpose(0, 3, 1, 4, 2, 5)
            .reshape(B, C, H, W))
    return np.ascontiguousarray(img, dtype=np.float32)
